# revision 1
# baseline (speedup 1.0000x reference)
"""AlexNet_flags Trainium2 kernel: data-parallel convs + model-parallel FC.

Layout conventions (per core, BL=32 images):
 - Conv activations in SBUF as [C_partitions, img, H+2p, W+2p] bf16, zero borders.
 - Conv = implicit GEMM: one matmul per kernel-offset accumulated into PSUM.
   K=128 achieved by pairing y-offsets: partitions 64-127 of each activation
   buffer hold a copy shifted by +1 row (y+1), so a single [128, N] rhs AP
   covers offsets (ky, kx) and (ky+1, kx) at once.
 - PSUM eviction fuses bias + ReLU (ACT engine), maxpool via 2x tensor_max.
 - FC: model-parallel over output features (512/core for fc1/fc2, 125/core
   for fc3) with an AllGather at each layer boundary.
"""
import os
import sys

sys.path.insert(0, "/opt/trn_rl_repo")
import numpy as np
import ml_dtypes

bf16 = ml_dtypes.bfloat16
f32np = np.float32
NCORES = 8
BL = 32  # images per core

_CACHE = {}


# ---------------------------------------------------------------- host prep
def _prep_shared(w):
    """Core-independent weight prep. w: dict of f32 arrays. Returns dict[str,np]."""
    out = {}
    w1, b1 = w["w1"], w["b1"]
    # conv1 im2col lhsT: row = (ky*3+kx)*3 + ci
    w1T = np.zeros((128, 128), f32np)
    w1T[0:27, 0:64] = w["w1"].transpose(2, 3, 1, 0).reshape(27, 64)
    out["w1T"] = w1T.astype(bf16)
    out["b1d"] = np.concatenate([b1, b1])[:, None].astype(f32np)  # [128,1]

    # conv2: 15 offset groups (dy in {0,2,4} paired with dy+1; dx 0..4)
    w2 = w["w2"]  # [192, 64, 5, 5]
    w2T = np.zeros((128, 15, 256), f32np)
    p = 0
    for dy in (0, 2, 4):
        for dx in range(5):
            blk = np.zeros((128, 192), f32np)
            blk[0:64] = w2[:, :, dy, dx].T
            if dy + 1 <= 4:
                blk[64:128] = w2[:, :, dy + 1, dx].T
            w2T[:, p, 0:128] = blk[:, 0:128]
            w2T[:, p, 128:192] = blk[:, 128:192]  # m1 zero-padded to 128
            p += 1
    out["w2T"] = w2T.astype(bf16)
    b2 = w["b2"]
    out["b2m0"] = b2[0:128, None].astype(f32np)
    out["b2m1"] = np.concatenate([b2[128:192], b2[128:192]])[:, None].astype(f32np)

    # conv3: full ktile (ci 0-127) 9 offsets; tail (ci 128-191) 6 paired groups
    w3 = w["w3"]  # [384, 192, 3, 3]
    w3T = np.zeros((128, 9, 384), f32np)
    for o, (ky, kx) in enumerate([(a, b) for a in range(3) for b in range(3)]):
        w3T[:, o, :] = w3[:, 0:128, ky, kx].T
    out["w3T"] = w3T.astype(bf16)
    w3Tt = np.zeros((128, 6, 384), f32np)
    for g, (ky, kx) in enumerate([(a, b) for a in (0, 2) for b in range(3)]):
        w3Tt[0:64, g, :] = w3[:, 128:192, ky, kx].T
        if ky + 1 <= 2:
            w3Tt[64:128, g, :] = w3[:, 128:192, ky + 1, kx].T
    out["w3Tt"] = w3Tt.astype(bf16)
    out["b3"] = w["b3"].reshape(3, 128).T.astype(f32np).copy()  # [128, 3]

    # conv4/conv5: full ktiles only
    def full_ktiles(wc, nkt):
        O = wc.shape[0]
        arr = np.zeros((128, nkt, 9, O), f32np)
        for kt in range(nkt):
            for o, (ky, kx) in enumerate(
                [(a, b) for a in range(3) for b in range(3)]
            ):
                arr[:, kt, o, :] = wc[:, 128 * kt : 128 * kt + 128, ky, kx].T
        return arr.astype(bf16)

    out["w4T"] = full_ktiles(w["w4"], 3)  # [128, 3, 9, 256]
    out["w5T"] = full_ktiles(w["w5"], 2)  # [128, 2, 9, 256]
    out["b4"] = w["b4"].reshape(2, 128).T.astype(f32np).copy()
    out["b5"] = w["b5"].reshape(2, 128).T.astype(f32np).copy()
    return out


def _prep_core(w, c):
    """Per-core FC weight slices."""
    out = {}
    fw1_sl = w["fw1"][512 * c : 512 * c + 512]  # [512, 4096]
    out["fw1T"] = np.ascontiguousarray(
        fw1_sl.reshape(4, 128, 32, 128).transpose(3, 2, 0, 1)
    ).astype(bf16)  # [128r, 32k, 4m, 128j]
    fw2_sl = w["fw2"][512 * c : 512 * c + 512]
    out["fw2T"] = np.ascontiguousarray(
        fw2_sl.reshape(4, 128, 32, 128).transpose(3, 2, 0, 1)
    ).astype(bf16)
    fw3_sl = w["fw3"][125 * c : 125 * c + 125]  # [125, 4096]
    out["fw3T"] = np.ascontiguousarray(
        fw3_sl.reshape(125, 32, 128).transpose(2, 1, 0)
    ).astype(bf16)  # [128, 32, 125]
    out["fb1"] = w["fb1"][512 * c : 512 * c + 512].reshape(4, 128).T.astype(f32np).copy()
    out["fb2"] = w["fb2"][512 * c : 512 * c + 512].reshape(4, 128).T.astype(f32np).copy()
    fb3 = np.zeros((128, 1), f32np)
    fb3[0:125, 0] = w["fb3"][125 * c : 125 * c + 125]
    out["fb3"] = fb3
    return out


OFFS9 = [(a, b) for a in range(3) for b in range(3)]
P15 = [(dy, dx) for dy in (0, 2, 4) for dx in range(5)]
T6 = [(ky, kx) for ky in (0, 2) for kx in range(3)]

# ---------------------------------------------------------------- builder
def _build(debug=False):
    import concourse.bacc as bacc
    import concourse.mybir as mybir
    from concourse.tile import TileContext

    dt = mybir.dt
    F32, BF = dt.float32, dt.bfloat16
    Relu = mybir.ActivationFunctionType.Relu
    ADD, MAX = mybir.AluOpType.add, mybir.AluOpType.max

    nc = bacc.Bacc("TRN2", target_bir_lowering=False, debug=False,
                   num_devices=NCORES)

    def din(name, shape, dtype=BF):
        return nc.dram_tensor(name, shape, dtype, kind="ExternalInput")

    x_pad = din("x_pad", [3, BL, 35, 36])
    w1T_d = din("w1T", [128, 128]); b1d_d = din("b1d", [128, 1], F32)
    w2T_d = din("w2T", [128, 15, 256])
    b2m0_d = din("b2m0", [128, 1], F32); b2m1_d = din("b2m1", [128, 1], F32)
    w3T_d = din("w3T", [128, 9, 384]); w3Tt_d = din("w3Tt", [128, 6, 384])
    b3_d = din("b3", [128, 3], F32)
    w4T_d = din("w4T", [128, 3, 9, 256]); b4_d = din("b4", [128, 2], F32)
    w5T_d = din("w5T", [128, 2, 9, 256]); b5_d = din("b5", [128, 2], F32)
    fw1T_d = din("fw1T", [128, 32, 4, 128]); fb1_d = din("fb1", [128, 4], F32)
    fw2T_d = din("fw2T", [128, 32, 4, 128]); fb2_d = din("fb2", [128, 4], F32)
    fw3T_d = din("fw3T", [128, 32, 125]); fb3_d = din("fb3", [128, 1], F32)
    yout = nc.dram_tensor("yout", [125, 256], F32, kind="ExternalOutput")
    dbg = {}
    if debug:
        def dout(name, shape, dtype=BF):
            dbg[name] = nc.dram_tensor(name, shape, dtype, kind="ExternalOutput")
            return dbg[name]
        d_a1 = dout("d_a1", [128, BL, 20, 20])
        d_a2m = dout("d_a2m", [128, BL, 10, 10])
        d_a2t = dout("d_a2t", [128, BL, 10, 10])
        d_a3 = dout("d_a3", [3, 128, BL, 10, 10])
        d_a4 = dout("d_a4", [2, 128, BL, 10, 10])
        d_a5 = dout("d_a5", [2, 128, 16, BL])
        d_H = dout("d_H", [128, 32, 256])
        d_H2 = dout("d_H2", [128, 32, 256])

    with TileContext(nc) as tc:
        ctxstack = []
        dma_engs = [nc.sync, nc.gpsimd, nc.scalar]
        _dmai = [0]
        def _dma(**kw):
            dma_engs[_dmai[0] % 3].dma_start(**kw)
            _dmai[0] += 1
        # persistent weight tiles
        wpool = tc.alloc_tile_pool(name="wts", bufs=1)
        ctxstack.append(wpool)
        def wt(dram, shape, dtype=BF):
            t = wpool.tile(shape, dtype, name=dram.name + "_t")
            _dma(out=t[...], in_=dram[...])
            return t
        w1T = wt(w1T_d, [128, 128]); b1d = wt(b1d_d, [128, 1], F32)
        w2T = wt(w2T_d, [128, 15, 256])
        b2m0 = wt(b2m0_d, [128, 1], F32); b2m1 = wt(b2m1_d, [128, 1], F32)

        # activations: one pool, tags recycled across layer generations
        acts = tc.alloc_tile_pool(name="acts", bufs=1)
        ctxstack.append(acts)
        a1 = acts.tile([128, BL, 20, 20], BF, name="a1", tag="g0")
        nc.vector.memset(a1[...], 0.0)

        pp = tc.alloc_tile_pool(name="ps", bufs=6, space="PSUM")
        ctxstack.append(pp)
        tpool = tc.alloc_tile_pool(name="tmps", bufs=3)
        ctxstack.append(tpool)

        # ---------------- conv1 (im2col K=27, col-tiled pair over y-halves)
        # xpad x-padded to 36 (pad 1 left, 3 right) + one guard row so each
        # (ky,kx) patch is one contiguous 32*36 flat block per image; junk in
        # patch columns 32-35 is never read by the matmul rhs (x sliced 0:32).
        xpf_d = x_pad[...].rearrange("p i y x -> p i (y x)")
        with tc.tile_pool(name="c1", bufs=1) as c1p:
            for g in range(2):
                patches = c1p.tile([128, 16, 32, 36], BF, name="patches",
                                   tag="patches", bufs=2)
                nc.gpsimd.memset(patches[:, 0:8, :, :], 0.0)
                nc.vector.memset(patches[:, 8:16, :, :], 0.0)
                paf = patches[...].rearrange("p i y x -> p i (y x)")
                for o, (ky, kx) in enumerate(OFFS9):
                    st = ky * 36 + kx
                    _dma(out=paf[3 * o:3 * o + 3, :, :],
                         in_=xpf_d[:, 16 * g:16 * g + 16, st:st + 32 * 36])
                for i in range(16):
                    I = 16 * g + i
                    # both y-halves in one col-paired psum; dedicated tag so
                    # conv2's psum slots aren't serialized behind conv1 evictions
                    ps = pp.tile([128, 512], F32, name="ps1", tag="ps1", bufs=2)
                    nc.tensor.matmul(ps[0:64, :], w1T[:, 0:64],
                                     patches[:, i, 0:16, 0:32],
                                     start=True, stop=True)
                    nc.tensor.matmul(ps[64:128, :], w1T[:, 0:64],
                                     patches[:, i, 16:32, 0:32],
                                     start=True, stop=True)
                    oc = tpool.tile([128, 16, 32], BF, name="oc", tag="oc")
                    nc.scalar.activation(
                        oc[...].rearrange("p y x -> p (y x)"),
                        ps[...], Relu, bias=b1d[:, 0:1])
                    t1 = tpool.tile([128, 16, 16], BF, name="t1", tag="t1")
                    nc.vector.tensor_max(t1[...], oc[:, :, 0::2], oc[:, :, 1::2])
                    nc.vector.tensor_max(
                        a1[0:64, I, 2:10, 2:18],
                        t1[0:64, 0::2, :], t1[0:64, 1::2, :])
                    t2 = tpool.tile([128, 8, 16], BF, name="t2", tag="t2")
                    nc.vector.tensor_max(t2[64:128, :, :],
                                         t1[64:128, 0::2, :], t1[64:128, 1::2, :])
                    _dma(out=a1[0:64, I, 10:18, 2:18], in_=t2[64:128, :, :])
                    # per-image y+1 dup for conv2 pairing, hidden behind compute
                    _dma(out=a1[64:128, I, 0:19, :], in_=a1[0:64, I, 1:20, :])
        w3T = wt(w3T_d, [128, 9, 384]); w3Tt = wt(w3Tt_d, [128, 6, 384])
        b3 = wt(b3_d, [128, 3], F32)
        w4T = wt(w4T_d, [128, 3, 9, 256]); b4 = wt(b4_d, [128, 2], F32)
        w5T = wt(w5T_d, [128, 2, 9, 256]); b5 = wt(b5_d, [128, 2], F32)

        # FC weights: loaded here so the DMA overlaps conv2-5 compute and the
        # allocation reuses the just-freed conv1 im2col space.
        fcwpool = tc.alloc_tile_pool(name="fcw", bufs=1)
        ctxstack.append(fcwpool)
        def wt2(dram, shape, dtype=BF):
            t = fcwpool.tile(shape, dtype, name=dram.name + "_t")
            nc.sync.dma_start(out=t[...], in_=dram[...])
            return t
        fw1T = wt2(fw1T_d, [128, 32, 4, 128]); fb1 = wt2(fb1_d, [128, 4], F32)
        fw2T = wt2(fw2T_d, [128, 32, 4, 128]); fb2 = wt2(fb2_d, [128, 4], F32)
        fw3T = wt2(fw3T_d, [128, 32, 125]); fb3 = wt2(fb3_d, [128, 1], F32)

        a2m = acts.tile([128, BL, 10, 10], BF, name="a2m", tag="g1")
        a2t = acts.tile([128, BL, 10, 10], BF, name="a2t", tag="g2")
        nc.gpsimd.memset(a2m[...], 0.0)
        nc.gpsimd.memset(a2t[...], 0.0)

        # ---------------- conv2 (5x5, 15 paired offset groups, pool)
        # m0: 128 output channels, full-mode
        for c in range(16):
            ps = pp.tile([128, 512], F32, name="ps", tag="ps")
            for p, (dy, dx) in enumerate(P15):
                nc.tensor.matmul(
                    ps[...], w2T[:, p, 0:128],
                    a1[:, 2 * c:2 * c + 2, dy:dy + 16, dx:dx + 16],
                    start=(p == 0), stop=(p == 14))
            tmp = tpool.tile([128, 2, 16, 16], BF, name="c2t", tag="c2t")
            nc.scalar.activation(
                tmp[...].rearrange("p a y x -> p (a y x)"),
                ps[...], Relu, bias=b2m0[:, 0:1])
            q1 = tpool.tile([128, 2, 16, 8], BF, name="q1", tag="q1")
            nc.vector.tensor_max(q1[...], tmp[:, :, :, 0::2], tmp[:, :, :, 1::2])
            nc.vector.tensor_max(a2m[:, 2 * c:2 * c + 2, 1:9, 1:9],
                                 q1[:, :, 0::2, :], q1[:, :, 1::2, :])
        # m1: 64 tail channels, col-paired: chunk 2j -> psum rows 0:64,
        # chunk 2j+1 -> rows 64:128 (concurrent col groups)
        for j in range(8):
            ps = pp.tile([128, 512], F32, name="ps", tag="ps")
            for p, (dy, dx) in enumerate(P15):
                nc.tensor.matmul(
                    ps[0:64, :], w2T[:, p, 128:192],
                    a1[:, 4 * j:4 * j + 2, dy:dy + 16, dx:dx + 16],
                    start=(p == 0), stop=(p == 14), skip_group_check=True)
                nc.tensor.matmul(
                    ps[64:128, :], w2T[:, p, 128:192],
                    a1[:, 4 * j + 2:4 * j + 4, dy:dy + 16, dx:dx + 16],
                    start=(p == 0), stop=(p == 14), skip_group_check=True)
            tmp = tpool.tile([128, 2, 16, 16], BF, name="c2t", tag="c2t")
            nc.scalar.activation(
                tmp[...].rearrange("p a y x -> p (a y x)"),
                ps[...], Relu, bias=b2m1[:, 0:1])
            q1 = tpool.tile([128, 2, 16, 8], BF, name="q1", tag="q1")
            nc.vector.tensor_max(q1[...], tmp[:, :, :, 0::2], tmp[:, :, :, 1::2])
            nc.vector.tensor_max(a2t[0:64, 4 * j:4 * j + 2, 1:9, 1:9],
                                 q1[0:64, :, 0::2, :], q1[0:64, :, 1::2, :])
            q2 = tpool.tile([128, 2, 8, 8], BF, name="q2", tag="q2")
            nc.vector.tensor_max(q2[64:128, :, :, :],
                                 q1[64:128, :, 0::2, :], q1[64:128, :, 1::2, :])
            for ii in range(2):
                _dma(out=a2t[0:64, 4 * j + 2 + ii, 1:9, 1:9],
                     in_=q2[64:128, ii, :, :])
            _dma(out=a2t[64:128, 4 * j:4 * j + 4, 0:9, :],
                 in_=a2t[0:64, 4 * j:4 * j + 4, 1:10, :])
        if debug:
            nc.sync.dma_start(out=dbg["d_a1"][...], in_=a1[...])
        a3 = []
        for i in range(3):
            t = acts.tile([128, BL, 10, 10], BF, name=f"a3_{i}", tag=f"g{3+i}")
            nc.gpsimd.memset(t[...], 0.0)
            a3.append(t)

        # ---------------- conv3 (K=192: 9 full + 6 paired tail groups)
        for m in range(3):
            for c in range(4):
                ps = pp.tile([128, 512], F32, name="ps", tag="ps")
                for o, (ky, kx) in enumerate(OFFS9):
                    nc.tensor.matmul(
                        ps[...], w3T[:, o, 128 * m:128 * m + 128],
                        a2m[:, 8 * c:8 * c + 8, ky:ky + 8, kx:kx + 8],
                        start=(o == 0), stop=False)
                for g, (ky, kx) in enumerate(T6):
                    nc.tensor.matmul(
                        ps[...], w3Tt[:, g, 128 * m:128 * m + 128],
                        a2t[:, 8 * c:8 * c + 8, ky:ky + 8, kx:kx + 8],
                        start=False, stop=(g == 5))
                nc.scalar.activation(
                    a3[m][:, 8 * c:8 * c + 8, 1:9, 1:9],
                    ps[...].rearrange("p (a y x) -> p a y x", a=8, y=8),
                    Relu, bias=b3[:, m:m + 1])

        if debug:
            nc.sync.dma_start(out=dbg["d_a2m"][...], in_=a2m[...])
            nc.sync.dma_start(out=dbg["d_a2t"][...], in_=a2t[...])
        a4 = []
        for i in range(2):
            t = acts.tile([128, BL, 10, 10], BF, name=f"a4_{i}", tag=f"g{1+i}")
            nc.gpsimd.memset(t[...], 0.0)
            a4.append(t)

        # ---------------- conv4 (K=384: 3 full ktiles)
        for m in range(2):
            for c in range(4):
                ps = pp.tile([128, 512], F32, name="ps", tag="ps")
                n = 0
                for kt in range(3):
                    for o, (ky, kx) in enumerate(OFFS9):
                        nc.tensor.matmul(
                            ps[...], w4T[:, kt, o, 128 * m:128 * m + 128],
                            a3[kt][:, 8 * c:8 * c + 8, ky:ky + 8, kx:kx + 8],
                            start=(n == 0), stop=(n == 26))
                        n += 1
                nc.scalar.activation(
                    a4[m][:, 8 * c:8 * c + 8, 1:9, 1:9],
                    ps[...].rearrange("p (a y x) -> p a y x", a=8, y=8),
                    Relu, bias=b4[:, m:m + 1])

        if debug:
            for i in range(3):
                nc.sync.dma_start(out=dbg["d_a3"][i], in_=a3[i][...])
        # a5 stored [C, px(4x4), img] = the bounce/H layout
        a5 = [acts.tile([128, 16, BL], BF, name=f"a5_{i}", tag=f"g{3+i}")
              for i in range(2)]

        # ---------------- conv5 (K=256) + pool -> a5; gather H per m-tile
        dpool = tc.alloc_tile_pool(name="dram", bufs=1, space="DRAM")
        ctxstack.append(dpool)
        H = acts.tile([128, 32, 256], BF, name="H", tag="g0")
        gathHs = []
        for m in range(2):
            for c in range(4):
                ps = pp.tile([128, 512], F32, name="ps", tag="ps")
                n = 0
                for kt in range(2):
                    for o, (ky, kx) in enumerate(OFFS9):
                        nc.tensor.matmul(
                            ps[...], w5T[:, kt, o, 128 * m:128 * m + 128],
                            a4[kt][:, 8 * c:8 * c + 8, ky:ky + 8, kx:kx + 8],
                            start=(n == 0), stop=(n == 17))
                        n += 1
                tmp = tpool.tile([128, 8, 8, 8], BF, name="c5t", tag="c5t")
                nc.scalar.activation(
                    tmp[...].rearrange("p a y x -> p (a y x)"),
                    ps[...], Relu, bias=b5[:, m:m + 1])
                q1 = tpool.tile([128, 8, 8, 4], BF, name="q5", tag="q5")
                nc.vector.tensor_max(q1[...], tmp[:, :, :, 0::2],
                                     tmp[:, :, :, 1::2])
                nc.vector.tensor_max(
                    a5[m][...].rearrange("c (y x) i -> c i y x", y=4)[
                        :, 8 * c:8 * c + 8, :, :],
                    q1[:, :, 0::2, :], q1[:, :, 1::2, :])
            bounceH = dpool.tile([128, 16, 32], BF, name=f"bounceH{m}")
            gathH = dpool.tile([NCORES, 128, 16, 32], BF, name=f"gathH{m}",
                               addr_space="Shared")
            nc.sync.dma_start(out=bounceH[...], in_=a5[m][...])
            nc.gpsimd.collective_compute(
                "AllGather", mybir.AluOpType.bypass,
                replica_groups=[list(range(NCORES))],
                ins=[bounceH.opt()], outs=[gathH.opt()])
            gathHs.append(gathH)
        for k in range(32):
            m, c0 = k // 16, 8 * (k % 16)
            dma_engs[k % 3].dma_start(
                out=H[:, k, :].rearrange("r (a i) -> r a i", a=NCORES),
                in_=gathHs[m][:, c0:c0 + 8, :, :].rearrange(
                    "a c p i -> (c p) a i"))
        if debug:
            for i in range(2):
                nc.sync.dma_start(out=dbg["d_a4"][i], in_=a4[i][...])
                nc.sync.dma_start(out=dbg["d_a5"][i], in_=a5[i][...])
            nc.sync.dma_start(out=dbg["d_H"][...], in_=H[...])

        # ---------------- fc1 / fc2: per-m AllGather pipeline
        H2 = acts.tile([128, 32, 256], BF, name="H2", tag="g6")
        H3 = acts.tile([128, 32, 256], BF, name="H3", tag="g0")
        for layer, (fwT, fb, Hin, Hout) in enumerate(
                [(fw1T, fb1, H, H2), (fw2T, fb2, H2, H3)]):
            # k-order m-major so ktiles gathered first are consumed first
            ks = (list(range(32)) if layer == 0 else
                  [4 * cc + mm for mm in range(4) for cc in range(8)])
            for m in range(4):
                ps = pp.tile([128, 256], F32, name="psfc", tag="ps")
                for j, k in enumerate(ks):
                    nc.tensor.matmul(ps[...], fwT[:, k, m, :], Hin[:, k, :],
                                     start=(j == 0), stop=(j == 31))
                hloc = tpool.tile([128, 256], BF, name="hloc", tag="hloc")
                nc.vector.tensor_scalar(hloc[...], ps[...],
                                        fb[:, m:m + 1], 0.0, ADD, MAX)
                bounce = dpool.tile([128, 256], BF, name=f"bnc{layer}_{m}")
                gath = dpool.tile([NCORES, 128, 256], BF,
                                  name=f"gath{layer}_{m}", addr_space="Shared")
                nc.sync.dma_start(out=bounce[...], in_=hloc[...])
                nc.gpsimd.collective_compute(
                    "AllGather", mybir.AluOpType.bypass,
                    replica_groups=[list(range(NCORES))],
                    ins=[bounce.opt()], outs=[gath.opt()])
                for cc in range(NCORES):
                    dma_engs[cc % 3].dma_start(out=Hout[:, 4 * cc + m, :],
                                               in_=gath[cc])
            if debug and layer == 0:
                nc.sync.dma_start(out=dbg["d_H2"][...], in_=Hout[...])

        # ---------------- fc3 (125 out-features per core, no relu)
        ks3 = [4 * cc + mm for mm in range(4) for cc in range(8)]
        psf = pp.tile([128, 256], F32, name="psf3", tag="ps")
        for j, k in enumerate(ks3):
            nc.tensor.matmul(psf[0:125, :], fw3T[:, k, :], H3[:, k, :],
                             start=(j == 0), stop=(j == 31))
        outt = acts.tile([128, 256], F32, name="outt", tag="g5")
        nc.vector.tensor_scalar(outt[0:125, :], psf[0:125, :],
                                fb3[0:125, 0:1], None, ADD)
        nc.sync.dma_start(out=yout[...], in_=outt[0:125, :])

        for p in reversed(ctxstack):
            p.release()

    nc.compile()
    return nc




def _run_pjrt_staged(nc, in_maps, n_cores):
    """run_bass_via_pjrt with inputs device_put ahead of the execute dispatch,
    so the 8 per-core launches land nearly simultaneously (less skew for the
    kernel's first collective to absorb)."""
    import jax
    import numpy as _np
    from jax.experimental.shard_map import shard_map
    from jax.sharding import Mesh, NamedSharding, PartitionSpec
    from concourse import bass2jax, mybir as _mybir

    bass2jax.install_neuronx_cc_hook()
    partition_name = (nc.partition_id_tensor.name
                      if nc.partition_id_tensor else None)
    in_names, out_names, out_avals, zero_outs = [], [], [], []
    for alloc in nc.m.functions[0].allocations:
        if not isinstance(alloc, _mybir.MemoryLocationSet):
            continue
        name = alloc.memorylocations[0].name
        if alloc.kind == "ExternalInput":
            if name != partition_name:
                in_names.append(name)
        elif alloc.kind == "ExternalOutput":
            out_names.append(name)
            shape = tuple(alloc.tensor_shape)
            dtype = _mybir.dt.np(alloc.dtype)
            out_avals.append(jax.core.ShapedArray(shape, dtype))
            zero_outs.append(_np.zeros(shape, dtype))
    n_params = len(in_names)
    n_outs = len(out_avals)
    param_names = list(in_names)
    in_names.extend(out_names)
    if partition_name is not None:
        in_names.append(partition_name)
    donate = tuple(range(n_params, n_params + n_outs))

    def _body(*args):
        operands = list(args)
        if partition_name is not None:
            operands.append(bass2jax.partition_id_tensor())
        outs = bass2jax._bass_exec_p.bind(
            *operands, out_avals=tuple(out_avals), in_names=tuple(in_names),
            out_names=tuple(out_names), lowering_input_output_aliases=(),
            sim_require_finite=True, sim_require_nnan=True, nc=nc)
        return tuple(outs)

    devices = jax.devices()[:n_cores]
    mesh = Mesh(_np.asarray(devices), ("core",))
    in_specs = (PartitionSpec("core"),) * (n_params + n_outs)
    out_specs = (PartitionSpec("core"),) * len(out_names)
    sharded = jax.jit(
        shard_map(_body, mesh=mesh, in_specs=in_specs, out_specs=out_specs,
                  check_rep=False),
        donate_argnums=donate, keep_unused=True)
    sh = NamedSharding(mesh, PartitionSpec("core"))
    concat_in = [
        _np.concatenate([_np.asarray(in_maps[c][nm]) for c in range(n_cores)],
                        axis=0)
        for nm in param_names
    ]
    concat_zeros = [
        _np.zeros((n_cores * z.shape[0], *z.shape[1:]), z.dtype)
        for z in zero_outs
    ]
    staged = [jax.device_put(a, sh) for a in concat_in + concat_zeros]
    jax.block_until_ready(staged)
    try:
        compiled = sharded.lower(*staged).compile()
        out_arrs = compiled(*staged)
    except Exception:
        out_arrs = sharded(*staged)
    return [
        {name: _np.asarray(out_arrs[i]).reshape(n_cores, *out_avals[i].shape)[c]
         for i, name in enumerate(out_names)}
        for c in range(n_cores)
    ]


# ---------------------------------------------------------------- entry
def _get_nc(debug=False):
    key = ("dbg" if debug else "rel")
    if key not in _CACHE:
        _CACHE[key] = _build(debug)
    return _CACHE[key]


def _make_in_maps(inputs):
    shared = _prep_shared(inputs)
    in_maps = []
    for c in range(NCORES):
        m = dict(shared)
        m.update(_prep_core(inputs, c))
        xs = inputs["x"][BL * c:BL * c + BL]  # [32, 3, 32, 32]
        xp = np.zeros((3, BL, 35, 36), f32np)
        xp[:, :, 1:33, 1:33] = xs.transpose(1, 0, 2, 3)
        m["x_pad"] = xp.astype(bf16)
        in_maps.append(m)
    return in_maps


class _StagedResult:
    def __init__(self, results):
        self.results = results
        self.exec_time_ns = None


def _run(inputs, debug=False, trace=False, **kw):
    nc = _get_nc(debug)
    in_maps = _make_in_maps(inputs)
    if trace:
        from concourse.bass_utils import run_bass_kernel_spmd
        return run_bass_kernel_spmd(nc, in_maps, core_ids=list(range(NCORES)),
                                    trace=True, **kw)
    try:
        return _StagedResult(_run_pjrt_staged(nc, in_maps, NCORES))
    except Exception:
        from concourse.bass_utils import run_bass_kernel_spmd
        return run_bass_kernel_spmd(nc, in_maps, core_ids=list(range(NCORES)),
                                    **kw)


def _unshard(results):
    out = np.zeros((256, 1000), f32np)
    for c in range(NCORES):
        out[:, 125 * c:125 * c + 125] = results[c]["yout"].T
    return out


def kernel(**inputs):
    inputs = {k: np.asarray(v) for k, v in inputs.items()}
    res = _run(inputs, debug=False)
    return _unshard(res.results)

